# revision 2
# baseline (speedup 1.0000x reference)
"""Bass/Trainium2 kernel for nn_DynamicNeuralGraph (gnn_message_passing).

Key reduction: the sequential edge scan h[tgt] += w * h[src] is a linear
operator on h0 along the neuron axis: h_final = M @ h0 with
M = L_{E-1} ... L_0, L_e = I + w_e * e_{tgt} e_{src}^T.  The output is
mean_n h_final[n] = v^T h0 with v^T = (1/N) * ones^T M, and v is computed
by propagating a row vector through the edges in reverse order
(u[src] += w * u[tgt]) -- 4096 scalar ops, done on host.

Since h0[n] = x @ W[n] + b[n]:
    out = x @ W_eff + b_eff,  W_eff = sum_n v_n W[n],  b_eff = v @ b

The only heavy device work is the v-weighted reduction of W (N,784,128)
= 102.8 MB read once -> memory-bound.  Sharding: each of the 8 cores
takes a 98-wide slice of the input dim I=784:
  stage 1: W_eff[islice,:] = sum_n v_n W[n,islice,:]   (PE matmul, v is the
           stationary operand (K=128, M=1), W streams as the moving operand,
           PSUM-accumulated over the two 128-neuron chunks)
  stage 2: partial out^T = W_eff[islice,:].T @ x[:,islice].T  (small GEMM)
Host sums the 8 partial (H,B) outputs and adds b_eff.
"""

import os

import numpy as np

N = 256      # neurons
I = 784      # input dim
H = 128      # hidden dim
B = 256      # batch
M_CORES = 8
ISL = I // M_CORES          # 98 i-rows per core
FD = ISL * H                # 12544 flat (i,h) elements per neuron row
NGROUP = 4                  # W streamed in 4 free-dim groups per n-chunk
GRP = FD // NGROUP          # 3136 elements per group (= 24.5 i-rows)
CH = 448                    # matmul free-dim chunk (<=512, fits PSUM bank)
NCH = GRP // CH             # 7 psum chunks per group
RHALF = FD // 2             # 6272 = 49 full i-rows (reshape granularity)

_compiled = None
_last_results = None  # for test harness introspection


def _build():
    import concourse.bacc as bacc
    import concourse.mybir as mybir
    import concourse.tile as tile

    nc = bacc.Bacc(
        "TRN2",
        target_bir_lowering=False,
        debug=False,
        num_devices=M_CORES,
    )
    f32 = mybir.dt.float32

    w = nc.dram_tensor("w", [N, FD], f32, kind="ExternalInput")
    vc = nc.dram_tensor("vc", [128, 2], f32, kind="ExternalInput")
    xt = nc.dram_tensor("xt", [ISL, B], f32, kind="ExternalInput")
    out = nc.dram_tensor("out", [H, B], f32, kind="ExternalOutput")

    with tile.TileContext(nc) as tc:
        with (
            tc.tile_pool(name="sb", bufs=1) as sb,
            tc.tile_pool(name="wp", bufs=2) as wp,
            tc.tile_pool(name="ps", bufs=7, space="PSUM") as ps,
            tc.tile_pool(name="ps2", bufs=1, space="PSUM") as ps2,
        ):
            vtile = sb.tile([128, 2], f32, tag="v")
            nc.sync.dma_start(vtile[:], vc[:])
            xtile = sb.tile([ISL, B], f32, tag="xt")
            nc.sync.dma_start(xtile[:], xt[:])

            # flat W_eff slice accumulates on partition 0, then two
            # reshape DMAs scatter it to (ISL, H) for the stage-2 GEMM
            wf = sb.tile([1, FD], f32, tag="wf")
            weff = sb.tile([ISL, H], f32, tag="weff")
            psum2 = ps2.tile([H, B], f32, tag="o")

            for g in range(NGROUP):
                a = wp.tile([128, GRP], f32, tag="A")
                bt = wp.tile([128, GRP], f32, tag="B")
                nc.sync.dma_start(a[:], w[0:128, g * GRP : (g + 1) * GRP])
                nc.sync.dma_start(bt[:], w[128:256, g * GRP : (g + 1) * GRP])
                for f in range(NCH):
                    p = ps.tile([1, CH], f32, tag="acc")
                    fs = slice(f * CH, (f + 1) * CH)
                    nc.tensor.matmul(
                        p[:], vtile[:, 0:1], a[:, fs], start=True, stop=False
                    )
                    nc.tensor.matmul(
                        p[:], vtile[:, 1:2], bt[:, fs], start=False, stop=True
                    )
                    o0 = g * GRP + f * CH
                    nc.vector.tensor_copy(wf[0:1, o0 : o0 + CH], p[:])
                if g % 2 == 1:
                    # 2 groups = 6272 elems = exactly 49 i-rows -> reshape
                    half = g // 2
                    rows = slice(half * 49, (half + 1) * 49)
                    nc.sync.dma_start(
                        weff[rows, :],
                        wf[0:1, half * RHALF : (half + 1) * RHALF],
                    )

            # stage 2: partial out^T = W_eff_slice.T @ x_slice.T  (K=98)
            nc.tensor.matmul(psum2[:], weff[:], xtile[:], start=True, stop=True)

            otile = sb.tile([H, B], f32, tag="ot")
            nc.vector.tensor_copy(otile[:], psum2[:])
            nc.sync.dma_start(out[:], otile[:])

    nc.compile()
    return nc


def _compute_v(edge_index, edge_weights):
    src = np.asarray(edge_index[0], dtype=np.int64)
    tgt = np.asarray(edge_index[1], dtype=np.int64)
    ew = np.asarray(edge_weights, dtype=np.float64)
    u = np.ones(N, dtype=np.float64)
    for e in range(ew.shape[0] - 1, -1, -1):
        u[src[e]] += ew[e] * u[tgt[e]]
    return (u / N).astype(np.float32)


def kernel(x, W, b, edge_index, edge_weights):
    global _compiled, _last_results
    from concourse.bass_utils import run_bass_kernel_spmd

    x = np.asarray(x, dtype=np.float32)
    W = np.asarray(W, dtype=np.float32)
    b = np.asarray(b, dtype=np.float32)

    v = _compute_v(edge_index, edge_weights)
    b_eff = v @ b  # (H,)
    vcols = np.ascontiguousarray(v.reshape(2, 128).T)  # [:, k] = chunk k

    if _compiled is None:
        _compiled = _build()

    in_maps = []
    for c in range(M_CORES):
        isl = slice(c * ISL, (c + 1) * ISL)
        wc = np.ascontiguousarray(W[:, isl, :]).reshape(N, FD)
        xtc = np.ascontiguousarray(x[:, isl].T)
        in_maps.append({"w": wc, "vc": vcols, "xt": xtc})

    trace = bool(int(os.environ.get("KERNEL_TRACE", "0")))
    res = run_bass_kernel_spmd(
        _compiled, in_maps, core_ids=list(range(M_CORES)), trace=trace
    )
    _last_results = res

    total = np.zeros((H, B), dtype=np.float32)
    for r in res.results:
        total += r["out"]
    return (total.T + b_eff[None, :]).astype(np.float32)


# revision 5
# speedup vs baseline: 1.0214x; 1.0214x over previous
"""Bass/Trainium2 kernel for nn_DynamicNeuralGraph (gnn_message_passing).

Key reduction: the sequential edge scan h[tgt] += w * h[src] is a linear
operator on h0 along the neuron axis: h_final = M @ h0 with
M = L_{E-1} ... L_0, L_e = I + w_e * e_{tgt} e_{src}^T.  The output is
mean_n h_final[n] = v^T h0 with v^T = (1/N) * ones^T M, and v is computed
by propagating a row vector through the edges in reverse order
(u[src] += w * u[tgt]) -- 4096 scalar ops, done on host.

Since h0[n] = x @ W[n] + b[n]:
    out = x @ W_eff + b_eff,  W_eff = sum_n v_n W[n],  b_eff = v @ b

The only heavy device work is the v-weighted reduction of W (N,784,128)
= 102.8 MB read once -> memory-bound.  Sharding: each of the 8 cores
takes a 98-wide slice of the input dim I=784.

fp32 matmuls on the TRN2 PE run 2 hardware passes (the fp32 stationary is
split), which made stage 1 PE-bound.  Instead W and v are decomposed on
the host into fp16 hi + 2048*residual fp16 lo (same total DMA bytes as
fp32, ~fp32 combined precision), and stage 1 runs four single-pass fp16
matmul streams:
    rows {vh,vl} x Wh  -> psum_hi (2, CH)   scales {1, 1/2048}
    rows {vh,vl} x Wl  -> psum_lo (2, CH)   scales {1/2048, 1/2048^2}
The PSUM->SBUF copies apply the row scales (DVE tensor_scalar for hi,
ACT activation for lo), giving four equal-scale flat W_eff components.
Reshape DMAs scatter them to (98,128) tiles and stage 2 accumulates
    out^T += weff_k.T @ x_slice^T        (4 fp32 matmuls, K=98)
Host sums the 8 partial (H,B) outputs and adds b_eff.
"""

import os

import numpy as np

N = 256      # neurons
I = 784      # input dim
H = 128      # hidden dim
B = 256      # batch
M_CORES = 8
ISL = I // M_CORES          # 98 i-rows per core
FD = ISL * H                # 12544 flat (i,h) elements per neuron row
NQ = 4                      # W streamed in 4 free-dim quarters per stream
QRT = FD // NQ              # 3136 elements per quarter
CH = 448                    # matmul free-dim chunk (fits one PSUM bank)
NCH = QRT // CH             # 7 psum chunks per quarter
RHALF = FD // 2             # 6272 = 49 full i-rows (reshape granularity)
LO_SCALE = 2048.0           # 2^11, exact in fp32
N_WARMUP = 20               # dummy bf16 matmuls to warm the PE HAM clock

_compiled = None
_last_results = None  # for test harness introspection


def _build():
    import concourse.bacc as bacc
    import concourse.mybir as mybir
    import concourse.tile as tile

    nc = bacc.Bacc(
        "TRN2",
        target_bir_lowering=False,
        debug=False,
        num_devices=M_CORES,
    )
    f32 = mybir.dt.float32
    f16 = mybir.dt.float16
    bf16 = mybir.dt.bfloat16

    wh = nc.dram_tensor("wh", [N, FD], f16, kind="ExternalInput")
    wl = nc.dram_tensor("wl", [N, FD], f16, kind="ExternalInput")
    vhl = nc.dram_tensor("vhl", [128, 4], f16, kind="ExternalInput")
    sc = nc.dram_tensor("sc", [2, 2], f32, kind="ExternalInput")
    xt = nc.dram_tensor("xt", [ISL, B], f32, kind="ExternalInput")
    out = nc.dram_tensor("out", [H, B], f32, kind="ExternalOutput")

    with tile.TileContext(nc) as tc:
        with (
            tc.tile_pool(name="sb", bufs=1) as sb,
            tc.tile_pool(name="wp", bufs=3) as wp,
            tc.tile_pool(name="psh", bufs=3, space="PSUM") as psh,
            tc.tile_pool(name="psl", bufs=3, space="PSUM") as psl,
            tc.tile_pool(name="ps2", bufs=1, space="PSUM") as ps2,
            tc.tile_pool(name="psw", bufs=1, space="PSUM") as psw,
        ):
            # PE warm-up: keep the HAM clock-gate busy while the first W
            # quarters stream in (no data deps -> scheduled immediately).
            junk = sb.tile([128, 640], bf16, tag="junk")
            nc.gpsimd.memset(junk[:], 0.0)
            pwarm = psw.tile([128, 512], f32, tag="warm")
            for _ in range(N_WARMUP):
                nc.tensor.matmul(
                    pwarm[:], junk[:, 512:640], junk[:, 0:512],
                    start=True, stop=True,
                )

            vtile = sb.tile([128, 4], f16, tag="v")
            nc.sync.dma_start(vtile[:], vhl[:])
            sctile = sb.tile([2, 2], f32, tag="sc")
            nc.sync.dma_start(sctile[:], sc[:])
            xtile = sb.tile([ISL, B], f32, tag="xt")
            nc.sync.dma_start(xtile[:], xt[:])

            # flat W_eff components accumulate on partitions 0-1, then
            # reshape DMAs scatter to (ISL, H) tiles for stage 2
            wf_hi = sb.tile([2, FD], f32, tag="wfh")
            wf_lo = sb.tile([2, FD], f32, tag="wfl")
            weff = [
                sb.tile([ISL, H], f32, tag=f"weff{k}", name=f"weff{k}")
                for k in range(4)
            ]
            psum2 = ps2.tile([H, B], f32, tag="o")

            for q in range(NQ):
                qs = slice(q * QRT, (q + 1) * QRT)
                ah = wp.tile([128, QRT], f16, tag="ah")
                bh = wp.tile([128, QRT], f16, tag="bh")
                al = wp.tile([128, QRT], f16, tag="al")
                bl = wp.tile([128, QRT], f16, tag="bl")
                nc.sync.dma_start(ah[:], wh[0:128, qs])
                nc.sync.dma_start(bh[:], wh[128:256, qs])
                nc.sync.dma_start(al[:], wl[0:128, qs])
                nc.sync.dma_start(bl[:], wl[128:256, qs])
                for f in range(NCH):
                    fs = slice(f * CH, (f + 1) * CH)
                    o0 = q * QRT + f * CH
                    ph = psh.tile([2, CH], f32, tag="acch")
                    pl = psl.tile([2, CH], f32, tag="accl")
                    nc.tensor.matmul(
                        ph[:], vtile[:, 0:2], ah[:, fs], start=True, stop=False
                    )
                    nc.tensor.matmul(
                        ph[:], vtile[:, 2:4], bh[:, fs], start=False, stop=True
                    )
                    nc.tensor.matmul(
                        pl[:], vtile[:, 0:2], al[:, fs], start=True, stop=False
                    )
                    nc.tensor.matmul(
                        pl[:], vtile[:, 2:4], bl[:, fs], start=False, stop=True
                    )
                    # scaled PSUM->SBUF copies, split across DVE and ACT
                    nc.vector.tensor_scalar_mul(
                        wf_hi[0:2, o0 : o0 + CH], ph[:], sctile[0:2, 0:1]
                    )
                    nc.scalar.activation(
                        wf_lo[0:2, o0 : o0 + CH],
                        pl[:],
                        mybir.ActivationFunctionType.Identity,
                        scale=sctile[0:2, 1:2],
                    )
                if q % 2 == 1:
                    # 2 quarters = 6272 elems = exactly 49 i-rows -> reshape
                    half = q // 2
                    rows = slice(half * 49, (half + 1) * 49)
                    hs = slice(half * RHALF, (half + 1) * RHALF)
                    nc.sync.dma_start(weff[0][rows, :], wf_hi[0:1, hs])
                    nc.sync.dma_start(weff[1][rows, :], wf_hi[1:2, hs])
                    nc.sync.dma_start(weff[2][rows, :], wf_lo[0:1, hs])
                    nc.sync.dma_start(weff[3][rows, :], wf_lo[1:2, hs])

            # stage 2: partial out^T = sum_k weff_k.T @ x_slice^T  (K=98)
            for k in range(4):
                nc.tensor.matmul(
                    psum2[:], weff[k][:], xtile[:],
                    start=(k == 0), stop=(k == 3),
                )

            otile = sb.tile([H, B], f32, tag="ot")
            nc.vector.tensor_copy(otile[:], psum2[:])
            nc.sync.dma_start(out[:], otile[:])

    nc.compile()
    return nc


def _compute_v(edge_index, edge_weights):
    src = np.asarray(edge_index[0], dtype=np.int64)
    tgt = np.asarray(edge_index[1], dtype=np.int64)
    ew = np.asarray(edge_weights, dtype=np.float64)
    u = np.ones(N, dtype=np.float64)
    for e in range(ew.shape[0] - 1, -1, -1):
        u[src[e]] += ew[e] * u[tgt[e]]
    return (u / N).astype(np.float32)


def _split_hi_lo(a):
    """a (fp32) -> (hi fp16, lo fp16) with a ~= hi + lo/LO_SCALE."""
    hi = a.astype(np.float16)
    lo = ((a - hi.astype(np.float32)) * LO_SCALE).astype(np.float16)
    return hi, lo


def kernel(x, W, b, edge_index, edge_weights):
    global _compiled, _last_results
    from concourse.bass_utils import run_bass_kernel_spmd

    x = np.asarray(x, dtype=np.float32)
    W = np.asarray(W, dtype=np.float32)
    b = np.asarray(b, dtype=np.float32)

    v = _compute_v(edge_index, edge_weights)
    b_eff = v @ b  # (H,)

    vh, vl = _split_hi_lo(v)
    # columns: [vh chunk0, vl chunk0, vh chunk1, vl chunk1]
    vhl = np.empty((128, 4), dtype=np.float16)
    vhl[:, 0] = vh[0:128]
    vhl[:, 1] = vl[0:128]
    vhl[:, 2] = vh[128:256]
    vhl[:, 3] = vl[128:256]
    s = np.float32(1.0 / LO_SCALE)
    # column 0 = hi-stream row scales, column 1 = lo-stream row scales
    sc = np.array([[1.0, s], [s, s * s]], dtype=np.float32)

    Wh, Wl = _split_hi_lo(W)

    if _compiled is None:
        _compiled = _build()

    in_maps = []
    for c in range(M_CORES):
        isl = slice(c * ISL, (c + 1) * ISL)
        whc = np.ascontiguousarray(Wh[:, isl, :]).reshape(N, FD)
        wlc = np.ascontiguousarray(Wl[:, isl, :]).reshape(N, FD)
        xtc = np.ascontiguousarray(x[:, isl].T)
        in_maps.append(
            {"wh": whc, "wl": wlc, "vhl": vhl, "sc": sc, "xt": xtc}
        )

    trace = bool(int(os.environ.get("KERNEL_TRACE", "0")))
    res = run_bass_kernel_spmd(
        _compiled, in_maps, core_ids=list(range(M_CORES)), trace=trace
    )
    _last_results = res

    total = np.zeros((H, B), dtype=np.float32)
    for r in res.results:
        total += r["out"]
    return (total.T + b_eff[None, :]).astype(np.float32)


# revision 9
# speedup vs baseline: 1.1697x; 1.1451x over previous
"""Bass/Trainium2 kernel for nn_DynamicNeuralGraph (gnn_message_passing).

Key reduction: the sequential edge scan h[tgt] += w * h[src] is a linear
operator on h0 along the neuron axis: h_final = M @ h0 with
M = L_{E-1} ... L_0, L_e = I + w_e * e_{tgt} e_{src}^T.  The output is
mean_n h_final[n] = v^T h0 with v^T = (1/N) * ones^T M, and v is computed
by propagating a row vector through the edges in reverse order
(u[src] += w * u[tgt]) -- 4096 scalar ops, done on host.

Since h0[n] = x @ W[n] + b[n]:
    out = x @ W_eff + b_eff,  W_eff = sum_n v_n W[n],  b_eff = v @ b

The only heavy device work is the v-weighted reduction of W (N,784,128)
= 102.8 MB read once -> memory-bound.  Sharding: each of the 8 cores
takes a 98-wide slice of the input dim I=784.

fp32 matmuls on the TRN2 PE run 2 hardware passes (the fp32 stationary is
split), which made stage 1 PE-bound.  Instead W and v are decomposed on
the host into fp16 hi + 2048*residual fp16 lo (same total DMA bytes as
fp32, ~fp32 combined precision), and stage 1 runs four single-pass fp16
matmul streams:
    rows {vh,vl} x Wh  -> psum_hi (2, CH)   scales {1, 1/2048}
    rows {vh,vl} x Wl  -> psum_lo (2, CH)   scales {1/2048, 1/2048^2}
The PSUM->SBUF copies apply the row scales (DVE tensor_scalar for hi,
ACT activation for lo), giving four equal-scale flat W_eff components.
Reshape DMAs scatter them to (98,128) tiles and stage 2 accumulates
    out^T += weff_k.T @ x_slice^T        (4 fp32 matmuls, K=98)
Host sums the 8 partial (H,B) outputs and adds b_eff.
"""

import os

import numpy as np

N = 256      # neurons
I = 784      # input dim
H = 128      # hidden dim
B = 256      # batch
M_CORES = 8
ISL = I // M_CORES          # 98 i-rows per core
FD = ISL * H                # 12544 flat (i,h) elements per neuron row
NQ = 4                      # W streamed in 4 free-dim quarters per stream
QRT = FD // NQ              # 3136 elements per quarter
CH = 448                    # matmul free-dim chunk (fits one PSUM bank)
NCH = QRT // CH             # 7 psum chunks per quarter
RHALF = FD // 2             # 6272 = 49 full i-rows (reshape granularity)
LO_SCALE = 2048.0           # 2^11, exact in fp32
N_WARMUP = 20               # dummy bf16 matmuls to warm the PE HAM clock

_compiled = None
_last_results = None  # for test harness introspection


def _build():
    import concourse.bacc as bacc
    import concourse.mybir as mybir
    import concourse.tile as tile

    nc = bacc.Bacc(
        "TRN2",
        target_bir_lowering=False,
        debug=False,
        num_devices=M_CORES,
    )
    f32 = mybir.dt.float32
    f16 = mybir.dt.float16
    bf16 = mybir.dt.bfloat16

    wh = nc.dram_tensor("wh", [N, FD], f16, kind="ExternalInput")
    wl = nc.dram_tensor("wl", [N, FD], f16, kind="ExternalInput")
    vhl = nc.dram_tensor("vhl", [128, 4], f16, kind="ExternalInput")
    sc = nc.dram_tensor("sc", [2, 2], f32, kind="ExternalInput")
    xt = nc.dram_tensor("xt", [ISL, B], f32, kind="ExternalInput")
    out = nc.dram_tensor("out", [H, B], f32, kind="ExternalOutput")

    with tile.TileContext(nc) as tc:
        with (
            tc.tile_pool(name="sb", bufs=1) as sb,
            tc.tile_pool(name="wp", bufs=3) as wp,
            tc.tile_pool(name="psh", bufs=3, space="PSUM") as psh,
            tc.tile_pool(name="psl", bufs=3, space="PSUM") as psl,
            tc.tile_pool(name="ps2", bufs=1, space="PSUM") as ps2,
            tc.tile_pool(name="psw", bufs=1, space="PSUM") as psw,
        ):
            # PE warm-up: keep the HAM clock-gate busy while the first W
            # quarters stream in (no data deps -> scheduled immediately).
            junk = sb.tile([128, 640], bf16, tag="junk")
            nc.vector.memset(junk[:], 0.0)
            pwarm = psw.tile([128, 512], f32, tag="warm")
            for _ in range(N_WARMUP):
                nc.tensor.matmul(
                    pwarm[:], junk[:, 512:640], junk[:, 0:512],
                    start=True, stop=True,
                )

            # small loads on the gpsimd SWDGE ring so the sync HWDGE ring
            # streams W from the first cycle
            vtile = sb.tile([128, 4], f16, tag="v")
            nc.gpsimd.dma_start(vtile[:], vhl[:])
            sctile = sb.tile([2, 2], f32, tag="sc")
            nc.gpsimd.dma_start(sctile[:], sc[:])
            xtile = sb.tile([ISL, B], f32, tag="xt")
            nc.gpsimd.dma_start(xtile[:], xt[:])

            # flat W_eff components accumulate on partitions 0-1, then
            # reshape DMAs scatter to (ISL, H) tiles for stage 2
            wf_hi = sb.tile([2, FD], f32, tag="wfh")
            wf_lo = sb.tile([2, FD], f32, tag="wfl")
            weff = [
                sb.tile([ISL, H], f32, tag=f"weff{k}", name=f"weff{k}")
                for k in range(4)
            ]
            psum2 = ps2.tile([H, B], f32, tag="o")

            for q in range(NQ):
                qs = slice(q * QRT, (q + 1) * QRT)
                ah = wp.tile([128, QRT], f16, tag="ah")
                bh = wp.tile([128, QRT], f16, tag="bh")
                al = wp.tile([128, QRT], f16, tag="al")
                bl = wp.tile([128, QRT], f16, tag="bl")
                nc.sync.dma_start(ah[:], wh[0:128, qs])
                nc.sync.dma_start(bh[:], wh[128:256, qs])
                nc.sync.dma_start(al[:], wl[0:128, qs])
                nc.sync.dma_start(bl[:], wl[128:256, qs])
                for f in range(NCH):
                    fs = slice(f * CH, (f + 1) * CH)
                    o0 = q * QRT + f * CH
                    ph = psh.tile([2, CH], f32, tag="acch")
                    pl = psl.tile([2, CH], f32, tag="accl")
                    # alternate PSUM banks between consecutive matmuls (same-
                    # bank pairs serialize on array drain) and reuse each
                    # stationary for two in a row (halves LDWEIGHTS count)
                    nc.tensor.matmul(
                        ph[:], vtile[:, 0:2], ah[:, fs], start=True, stop=False
                    )
                    nc.tensor.matmul(
                        pl[:], vtile[:, 0:2], al[:, fs], start=True, stop=False
                    )
                    nc.tensor.matmul(
                        ph[:], vtile[:, 2:4], bh[:, fs], start=False, stop=True
                    )
                    nc.tensor.matmul(
                        pl[:], vtile[:, 2:4], bl[:, fs], start=False, stop=True
                    )
                    # scaled PSUM->SBUF copies, split across DVE and ACT
                    nc.vector.tensor_scalar_mul(
                        wf_hi[0:2, o0 : o0 + CH], ph[:], sctile[0:2, 0:1]
                    )
                    nc.scalar.activation(
                        wf_lo[0:2, o0 : o0 + CH],
                        pl[:],
                        mybir.ActivationFunctionType.Identity,
                        scale=sctile[0:2, 1:2],
                    )
                if q % 2 == 1:
                    # 2 quarters = 6272 elems = exactly 49 i-rows -> reshape
                    half = q // 2
                    rows = slice(half * 49, (half + 1) * 49)
                    hs = slice(half * RHALF, (half + 1) * RHALF)
                    # separate SWDGE ring: don't steal HWDGE ring time from
                    # the W stream with these single-partition-source copies
                    nc.gpsimd.dma_start(weff[0][rows, :], wf_hi[0:1, hs])
                    nc.gpsimd.dma_start(weff[1][rows, :], wf_hi[1:2, hs])
                    nc.gpsimd.dma_start(weff[2][rows, :], wf_lo[0:1, hs])
                    nc.gpsimd.dma_start(weff[3][rows, :], wf_lo[1:2, hs])

            # stage 2: partial out^T = sum_k weff_k.T @ x_slice^T  (K=98)
            for k in range(4):
                nc.tensor.matmul(
                    psum2[:], weff[k][:], xtile[:],
                    start=(k == 0), stop=(k == 3),
                )

            otile = sb.tile([H, B], f32, tag="ot")
            nc.vector.tensor_copy(otile[:], psum2[:])
            nc.sync.dma_start(out[:], otile[:])

    nc.compile()
    return nc


def _compute_v(edge_index, edge_weights):
    src = np.asarray(edge_index[0], dtype=np.int64)
    tgt = np.asarray(edge_index[1], dtype=np.int64)
    ew = np.asarray(edge_weights, dtype=np.float64)
    u = np.ones(N, dtype=np.float64)
    for e in range(ew.shape[0] - 1, -1, -1):
        u[src[e]] += ew[e] * u[tgt[e]]
    return (u / N).astype(np.float32)


def _split_hi_lo(a):
    """a (fp32) -> (hi fp16, lo fp16) with a ~= hi + lo/LO_SCALE."""
    hi = a.astype(np.float16)
    lo = ((a - hi.astype(np.float32)) * LO_SCALE).astype(np.float16)
    return hi, lo


def kernel(x, W, b, edge_index, edge_weights):
    global _compiled, _last_results
    from concourse.bass_utils import run_bass_kernel_spmd

    x = np.asarray(x, dtype=np.float32)
    W = np.asarray(W, dtype=np.float32)
    b = np.asarray(b, dtype=np.float32)

    v = _compute_v(edge_index, edge_weights)
    b_eff = v @ b  # (H,)

    vh, vl = _split_hi_lo(v)
    # columns: [vh chunk0, vl chunk0, vh chunk1, vl chunk1]
    vhl = np.empty((128, 4), dtype=np.float16)
    vhl[:, 0] = vh[0:128]
    vhl[:, 1] = vl[0:128]
    vhl[:, 2] = vh[128:256]
    vhl[:, 3] = vl[128:256]
    s = np.float32(1.0 / LO_SCALE)
    # column 0 = hi-stream row scales, column 1 = lo-stream row scales
    sc = np.array([[1.0, s], [s, s * s]], dtype=np.float32)

    Wh, Wl = _split_hi_lo(W)

    if _compiled is None:
        _compiled = _build()

    in_maps = []
    for c in range(M_CORES):
        isl = slice(c * ISL, (c + 1) * ISL)
        whc = np.ascontiguousarray(Wh[:, isl, :]).reshape(N, FD)
        wlc = np.ascontiguousarray(Wl[:, isl, :]).reshape(N, FD)
        xtc = np.ascontiguousarray(x[:, isl].T)
        in_maps.append(
            {"wh": whc, "wl": wlc, "vhl": vhl, "sc": sc, "xt": xtc}
        )

    trace = bool(int(os.environ.get("KERNEL_TRACE", "0")))
    res = run_bass_kernel_spmd(
        _compiled, in_maps, core_ids=list(range(M_CORES)), trace=trace
    )
    _last_results = res

    total = np.zeros((H, B), dtype=np.float32)
    for r in res.results:
        total += r["out"]
    return (total.T + b_eff[None, :]).astype(np.float32)
